# revision 21
# baseline (speedup 1.0000x reference)
"""Cross-attention kernel for 8 TRN2 NeuronCores (v5, bf16).

Problem: B=4, T_V=8192, T_T=77, C=1024, H=16, D=64 (f32 in/out).
  q = video @ Wq.T ; k,v = text @ W.T ; out = softmax(qk/sqrt(D)) v @ Wo.T

Sharding: data-parallel over (batch, T_V/2) -> 8 shards of [4096, 1024].
Each core gets its video shard, its batch's text, and all weights.
No collectives.

Design highlights:
  * everything bf16 on the wire and in SBUF (host casts inputs/weights,
    output returns bf16 and the host upcasts) -> half the DMA bytes,
    full-rate matmuls, cheap DVE/ACT ops.
  * whole-tensor weight DMAs, spread across both HWDGE rings.
  * softmax denominators come from 16 accumulating "selector" matmuls
    (stationary sel_h = e_h outer ones_77) into one PSUM tile
    psden[16,512], then ONE bit-exact DVE reciprocal [16,512] per block
    (the DVE approx-reciprocal custom op mis-executes on HW; the ACT
    Reciprocal table thrashes against Exp), 8 small per-jc
    partition-broadcast DMAs, and 8 full-width [128,512] bf16
    multiplies.
  * T padded 77->80; pad K^T columns are zero => pad scores 0 =>
    exp()=1, but V pad rows and selector pad rows are zeroed so the
    pads contribute nothing to O' or the denominators.

On-chip dataflow (all "transposed": rows of video on the FREE dim):
  host pre-transposes X -> X^T [C, M] and weights -> W^T [C, C] so the
  contraction dim always lands on SBUF partitions.
  Q^T = WqT-chunks . X^T ; K^T [C, T] from text; V natural [T, C].
  per block: 16x (S^T_h = K_h^T . Q_h^T -> exp on ScalarE, scale=1/8
  folded in, no max-subtraction: scores are O(1) bounded), then 16x
  (O'^T_h = V_h . expS^T_h ; psden += sel_h . expS^T_h), reciprocal,
  broadcast, normalize into ot^T.
  out = O^T-chunks . WoT in natural [m, n] layout via PSUM -> bf16 ob
  tiles -> DRAM.
"""

import sys

if "/opt/trn_rl_repo" not in sys.path:
    sys.path.insert(0, "/opt/trn_rl_repo")

import numpy as np

import concourse.bacc as bacc
import concourse.mybir as mybir
import concourse.tile as tile
from concourse.bass_utils import run_bass_kernel_spmd

F32 = mybir.dt.float32
BF16 = mybir.dt.bfloat16
AF = mybir.ActivationFunctionType
MULT = mybir.AluOpType.mult

# This kernel's only table-based ACT funcs are Exp and Ln. act_info.json's
# "natural_log_exp_and_others" set contains both, but bacc's table-load
# insertion maps each func to the first set containing it, so Exp resolves
# to "exp_and_others" and every Exp<->Ln alternation costs a ~1.3us table
# reload (128+ per kernel). Narrow the pass's view so Exp only matches the
# set that also holds Ln; the emitted set ids still point at the real
# act_info.json sets, so runtime tables stay correct.
import concourse.hw_specs as _hw_specs
import concourse.bacc as _bacc_mod

_orig_get_act_tables = _hw_specs.get_activation_tables


def _patched_get_act_tables(arch):
    tabs = {k: set(v) for k, v in _orig_get_act_tables(arch).items()}
    for name, funcs in tabs.items():
        if AF.Ln not in funcs:
            funcs.discard(AF.Exp)
    return tabs


_hw_specs.get_activation_tables = _patched_get_act_tables
_bacc_mod.get_activation_tables = _patched_get_act_tables

B, T_V, T_T, C, H = 4, 8192, 77, 1024, 16
D = C // H            # 64
P = 128
KC = C // P           # 8 contraction chunks
M = T_V // 2          # 4096 rows per core
MB = 512              # m-block (rows processed per pipeline stage)
NBLK = M // MB        # 8
MSUB = MB // P        # 4 output row-chunks per block
T = T_T               # 77
TP = 80               # padded T
DV = D + 2            # 66: 64 head dims + ones col + pad col (even)
SCALE = 1.0 / float(np.sqrt(D))

_CACHED_NC = None


def _build(repeat: int = 1, nblk: int = NBLK):
    nc = bacc.Bacc(name="cross_attention_v5")

    xt = nc.dram_tensor("xt", [C, M], BF16, kind="ExternalInput")
    yt = nc.dram_tensor("yt", [C, T], BF16, kind="ExternalInput")
    wqt = nc.dram_tensor("wqt", [C, C], BF16, kind="ExternalInput")
    wkt = nc.dram_tensor("wkt", [C, C], BF16, kind="ExternalInput")
    wvt = nc.dram_tensor("wvt", [C, C], BF16, kind="ExternalInput")
    wot = nc.dram_tensor("wot", [C, C], BF16, kind="ExternalInput")
    out = nc.dram_tensor("out", [M, C], BF16, kind="ExternalOutput")

    # [C, X] dram views chunked to [P, KC, X]
    xt_v = xt[:, :].rearrange("(kc p) m -> p kc m", p=P)
    yt_v = yt[:, :].rearrange("(kc p) t -> p kc t", p=P)
    wq_v = wqt[:, :].rearrange("(kc p) n -> p kc n", p=P)
    wk_v = wkt[:, :].rearrange("(kc p) n -> p kc n", p=P)
    wv_v = wvt[:, :].rearrange("(kc p) n -> p kc n", p=P)
    wo_v = wot[:, :].rearrange("(kc p) n -> p kc n", p=P)

    with tile.TileContext(nc) as tc:
        with (
            tc.tile_pool(name="wq", bufs=1) as wq_pool,
            tc.tile_pool(name="wo", bufs=1) as wo_pool,
            tc.tile_pool(name="kt", bufs=1) as kt_pool,
            tc.tile_pool(name="vv", bufs=1) as v_pool,
        ):
            wq_sb = wq_pool.tile([P, KC, C], BF16)
            wo_sb = wo_pool.tile([P, KC, C], BF16)
            kt_sb = kt_pool.tile([P, KC, TP], BF16)
            v_sb = v_pool.tile([TP, H, DV], BF16)

            nc.sync.dma_start(wq_sb[:], wq_v[:])

            # ---- prologue: K^T and V from text; load Wk/Wv/Wo ----
            with (
                tc.tile_pool(name="wkv", bufs=1) as wkv_pool,
                tc.tile_pool(name="yt", bufs=1) as yt_pool,
                tc.tile_pool(name="pspro", bufs=2, space="PSUM") as ps_pro,
            ):
                yt_sb = yt_pool.tile([P, KC, TP], BF16)
                nc.vector.memset(yt_sb[:], 0.0)
                nc.sync.dma_start(yt_sb[:, :, :T], yt_v[:])

                wk_sb = wkv_pool.tile([P, KC, C], BF16, tag="wkv")
                nc.scalar.dma_start(wk_sb[:], wk_v[:])
                # K^T [C, T]: chunk nc_ holds rows 128*nc_..128*nc_+128
                for nc_ in range(KC):
                    psk_full = ps_pro.tile([P, MB], F32, tag="pro", name="psk")
                    psk = psk_full[:, :TP]
                    for kc in range(KC):
                        nc.tensor.matmul(
                            psk[:],
                            wk_sb[:, kc, nc_ * P : (nc_ + 1) * P],
                            yt_sb[:, kc, :],
                            start=(kc == 0),
                            stop=(kc == KC - 1),
                        )
                    nc.vector.tensor_copy(out=kt_sb[:, nc_, :], in_=psk[:])

                wv_sb = wkv_pool.tile([P, KC, C], BF16, tag="wkv")
                nc.scalar.dma_start(wv_sb[:], wv_v[:])
                # V natural [T, C] written per 512-wide column slab into
                # the strided per-head layout v_sb[t, h, 0:64]; pad rows
                # 77:80 stay zero.
                nc.vector.memset(v_sb[:], 0.0)
                for half in range(2):
                    psv_full = ps_pro.tile([P, MB], F32, tag="pro", name="psv")
                    psv = psv_full[:T, :]
                    for kc in range(KC):
                        nc.tensor.matmul(
                            psv[:],
                            yt_sb[:, kc, :T],
                            wv_sb[:, kc, half * MB : (half + 1) * MB],
                            start=(kc == 0),
                            stop=(kc == KC - 1),
                        )
                    nc.vector.tensor_copy(
                        out=v_sb[:T, half * 8 : (half + 1) * 8, 0:D],
                        in_=psv[:].rearrange("t (h d) -> t h d", d=D),
                    )
                nc.vector.memset(v_sb[:T, :, D : D + 1], 1.0)

            nc.scalar.dma_start(wo_sb[:], wo_v[:])

            # ---- main pipeline over m-blocks ----
            from contextlib import ExitStack

            with ExitStack() as _st:
                xt_pool = _st.enter_context(tc.tile_pool(name="xt", bufs=2))
                qt_pool = _st.enter_context(tc.tile_pool(name="qt", bufs=2))
                ot_pool = _st.enter_context(tc.tile_pool(name="ot", bufs=2))
                ou_pool = _st.enter_context(tc.tile_pool(name="ou", bufs=2))
                es_pool = _st.enter_context(tc.tile_pool(name="es", bufs=2))
                rc_pool = _st.enter_context(tc.tile_pool(name="rc", bufs=2))
                ln_pool = _st.enter_context(tc.tile_pool(name="ln", bufs=3))
                rb_pool = _st.enter_context(tc.tile_pool(name="rb", bufs=2))
                ob_pool = _st.enter_context(tc.tile_pool(name="ob", bufs=3))
                ps_q = _st.enter_context(tc.tile_pool(name="psq", bufs=3, space="PSUM"))
                ps_s = _st.enter_context(tc.tile_pool(name="pss", bufs=1, space="PSUM"))
                ps_o = _st.enter_context(tc.tile_pool(name="pso", bufs=1, space="PSUM"))
                ps_out = _st.enter_context(tc.tile_pool(name="psout", bufs=3, space="PSUM"))
                for j in [jj for _ in range(repeat) for jj in range(nblk)]:
                    xt_t = xt_pool.tile([P, KC, MB], BF16, tag="xt")
                    nc.sync.dma_start(
                        xt_t[:], xt_v[:, :, j * MB : (j + 1) * MB]
                    )

                    # Q^T chunks for this block
                    qt_t = qt_pool.tile([P, KC, MB], BF16, tag="qt")
                    for nc_ in range(KC):
                        psq = ps_q.tile([P, MB], F32, tag="psq")
                        for kc in range(KC):
                            nc.tensor.matmul(
                                psq[:],
                                wq_sb[:, kc, nc_ * P : (nc_ + 1) * P],
                                xt_t[:, kc, :],
                                start=(kc == 0),
                                stop=(kc == KC - 1),
                            )
                        nc.vector.tensor_copy(out=qt_t[:, nc_, :], in_=psq[:])

                    # attention phase 1: scores + exp for all heads
                    es_t = es_pool.tile([TP, H, MB], BF16, tag="es")
                    for h in range(H):
                        jc, hf = divmod(h, 2)
                        lo, hi = 64 * hf, 64 * hf + 64
                        pss = ps_s.tile([TP, MB], F32, tag="pss")
                        nc.tensor.matmul(
                            pss[:],
                            kt_sb[lo:hi, jc, :],
                            qt_t[lo:hi, jc, :],
                            start=True,
                            stop=True,
                        )
                        nc.scalar.activation(
                            es_t[:, h, :], pss[:], AF.Exp, scale=SCALE
                        )

                    # phase 2: AV into ou_t (unnormalized O'), selector
                    # matmuls accumulate all 16 denominators into psden.
                    ot_t = ot_pool.tile([P, KC, MB], BF16, tag="ot")
                    ou_t = ou_pool.tile([P, KC, MB], BF16, tag="ou")
                    # rcb[0, hf, jc, :] = 1/den of head h = 2*jc + hf via
                    # exp(-ln(den)): Ln/Exp share one ACT table set (see
                    # the get_activation_tables patch above), so no table
                    # reloads.
                    rcb = rc_pool.tile([1, 2, KC, MB], BF16, tag="rc")
                    for h in range(H):
                        jc, hf = divmod(h, 2)
                        lo, hi = 64 * hf, 64 * hf + 64
                        pso = ps_o.tile([DV, MB], F32, tag="pso")
                        nc.tensor.matmul(
                            pso[:], v_sb[:, h, :], es_t[:, h, :],
                            start=True, stop=True,
                        )
                        # softmax denominator lives in PSUM row D
                        lnd = ln_pool.tile([1, MB], F32, tag="lnd")
                        nc.scalar.activation(
                            lnd[:], pso[D : D + 1, :], AF.Ln
                        )
                        nc.scalar.activation(
                            rcb[:, hf, jc, :], lnd[:],
                            AF.Exp, scale=-1.0,
                        )
                        nc.vector.tensor_copy(
                            out=ou_t[lo:hi, jc, :], in_=pso[0:D, :]
                        )

                    # batched partition-broadcast of the reciprocals:
                    # rb[64*hf + p, jc, m] = rcb[0, hf, jc, m]
                    rb = rb_pool.tile([P, KC, MB], BF16, tag="rb")
                    for hf in range(2):
                        nc.scalar.dma_start(
                            rb[64 * hf : 64 * hf + 64, :, :],
                            rcb[0:1, hf, None, :, :].to_broadcast(
                                (1, 64, KC, MB)
                            ),
                        )
                    for jc in range(KC):
                        nc.vector.tensor_tensor(
                            ot_t[:, jc, :],
                            ou_t[:, jc, :],
                            rb[:, jc, :],
                            MULT,
                        )

                    # output projection, natural [m, n] layout
                    for mi in range(MSUB):
                        ob = ob_pool.tile([P, 2, MB], BF16, tag="ob")
                        for nh in range(2):
                            pst = ps_out.tile([P, MB], F32, tag="psout")
                            for cc in range(KC):
                                nc.tensor.matmul(
                                    pst[:],
                                    ot_t[:, cc, mi * P : (mi + 1) * P],
                                    wo_sb[:, cc, nh * MB : (nh + 1) * MB],
                                    start=(cc == 0),
                                    stop=(cc == KC - 1),
                                )
                            nc.vector.tensor_copy(out=ob[:, nh, :], in_=pst[:])
                        nc.sync.dma_start(
                            out[j * MB + mi * P : j * MB + (mi + 1) * P, :],
                            ob[:],
                        )
    nc.finalize()
    return nc


def _get_nc(repeat: int = 1):
    global _CACHED_NC
    if _CACHED_NC is None:
        _CACHED_NC = {}
    if repeat not in _CACHED_NC:
        _CACHED_NC[repeat] = _build(repeat)
    return _CACHED_NC[repeat]


def _np_bf16():
    return np.dtype(mybir.dt.np(BF16))


def make_in_maps(video_features, text_features, Wq, Wk, Wv, Wo):
    bf16 = _np_bf16()
    video_features = np.asarray(video_features, dtype=np.float32)
    text_features = np.asarray(text_features, dtype=np.float32)
    wqt = np.ascontiguousarray(np.asarray(Wq, dtype=np.float32).T).astype(bf16)
    wkt = np.ascontiguousarray(np.asarray(Wk, dtype=np.float32).T).astype(bf16)
    wvt = np.ascontiguousarray(np.asarray(Wv, dtype=np.float32).T).astype(bf16)
    wot = np.ascontiguousarray(np.asarray(Wo, dtype=np.float32).T).astype(bf16)

    in_maps = []
    for c in range(8):
        b, half = divmod(c, 2)
        xs = video_features[b, half * M : (half + 1) * M, :]  # [M, C]
        in_maps.append(
            {
                "xt": np.ascontiguousarray(xs.T).astype(bf16),    # [C, M]
                "yt": np.ascontiguousarray(text_features[b].T).astype(bf16),
                "wqt": wqt,
                "wkt": wkt,
                "wvt": wvt,
                "wot": wot,
            }
        )
    return in_maps


def kernel(video_features, text_features, Wq, Wk, Wv, Wo, **_unused):
    in_maps = make_in_maps(video_features, text_features, Wq, Wk, Wv, Wo)
    res = run_bass_kernel_spmd(_get_nc(), in_maps, core_ids=list(range(8)))
    outf = np.empty((B, T_V, C), dtype=np.float32)
    for c in range(8):
        b, half = divmod(c, 2)
        outf[b, half * M : (half + 1) * M, :] = (
            res.results[c]["out"].astype(np.float32)
        )
    return outf


# revision 22
# speedup vs baseline: 1.4561x; 1.4561x over previous
"""Cross-attention kernel for 8 TRN2 NeuronCores (v5, bf16).

Problem: B=4, T_V=8192, T_T=77, C=1024, H=16, D=64 (f32 in/out).
  q = video @ Wq.T ; k,v = text @ W.T ; out = softmax(qk/sqrt(D)) v @ Wo.T

Sharding: data-parallel over (batch, T_V/2) -> 8 shards of [4096, 1024].
Each core gets its video shard, its batch's text, and all weights.
No collectives.

Design highlights:
  * everything bf16 on the wire and in SBUF (host casts inputs/weights,
    output returns bf16 and the host upcasts) -> half the DMA bytes,
    full-rate matmuls, cheap DVE/ACT ops.
  * whole-tensor weight DMAs, spread across both HWDGE rings.
  * softmax denominators come from 16 accumulating "selector" matmuls
    (stationary sel_h = e_h outer ones_77) into one PSUM tile
    psden[16,512], then ONE bit-exact DVE reciprocal [16,512] per block
    (the DVE approx-reciprocal custom op mis-executes on HW; the ACT
    Reciprocal table thrashes against Exp), 8 small per-jc
    partition-broadcast DMAs, and 8 full-width [128,512] bf16
    multiplies.
  * T padded 77->80; pad K^T columns are zero => pad scores 0 =>
    exp()=1, but V pad rows and selector pad rows are zeroed so the
    pads contribute nothing to O' or the denominators.

On-chip dataflow (all "transposed": rows of video on the FREE dim):
  host pre-transposes X -> X^T [C, M] and weights -> W^T [C, C] so the
  contraction dim always lands on SBUF partitions.
  Q^T = WqT-chunks . X^T ; K^T [C, T] from text; V natural [T, C].
  per block: 16x (S^T_h = K_h^T . Q_h^T -> exp on ScalarE, scale=1/8
  folded in, no max-subtraction: scores are O(1) bounded), then 16x
  (O'^T_h = V_h . expS^T_h ; psden += sel_h . expS^T_h), reciprocal,
  broadcast, normalize into ot^T.
  out = O^T-chunks . WoT in natural [m, n] layout via PSUM -> bf16 ob
  tiles -> DRAM.
"""

import sys

if "/opt/trn_rl_repo" not in sys.path:
    sys.path.insert(0, "/opt/trn_rl_repo")

import numpy as np

import concourse.bacc as bacc
import concourse.mybir as mybir
import concourse.tile as tile
from concourse.bass_utils import run_bass_kernel_spmd

F32 = mybir.dt.float32
BF16 = mybir.dt.bfloat16
AF = mybir.ActivationFunctionType
MULT = mybir.AluOpType.mult

# This kernel's only table-based ACT funcs are Exp and Ln. act_info.json's
# "natural_log_exp_and_others" set contains both, but bacc's table-load
# insertion maps each func to the first set containing it, so Exp resolves
# to "exp_and_others" and every Exp<->Ln alternation costs a ~1.3us table
# reload (128+ per kernel). Narrow the pass's view so Exp only matches the
# set that also holds Ln; the emitted set ids still point at the real
# act_info.json sets, so runtime tables stay correct.
import concourse.hw_specs as _hw_specs
import concourse.bacc as _bacc_mod

_orig_get_act_tables = _hw_specs.get_activation_tables


def _patched_get_act_tables(arch):
    tabs = {k: set(v) for k, v in _orig_get_act_tables(arch).items()}
    for name, funcs in tabs.items():
        if AF.Ln not in funcs:
            funcs.discard(AF.Exp)
    return tabs


_hw_specs.get_activation_tables = _patched_get_act_tables
_bacc_mod.get_activation_tables = _patched_get_act_tables

B, T_V, T_T, C, H = 4, 8192, 77, 1024, 16
D = C // H            # 64
P = 128
KC = C // P           # 8 contraction chunks
M = T_V // 2          # 4096 rows per core
MB = 512              # m-block (rows processed per pipeline stage)
NBLK = M // MB        # 8
MSUB = MB // P        # 4 output row-chunks per block
T = T_T               # 77
TP = 80               # padded T
DV = D + 2            # 66: 64 head dims + ones col + pad col (even)
SCALE = 1.0 / float(np.sqrt(D))

_CACHED_NC = None


def _build(repeat: int = 1, nblk: int = NBLK):
    nc = bacc.Bacc(name="cross_attention_v5")

    xt = nc.dram_tensor("xt", [C, M], BF16, kind="ExternalInput")
    yt = nc.dram_tensor("yt", [C, T], BF16, kind="ExternalInput")
    wqt = nc.dram_tensor("wqt", [C, C], BF16, kind="ExternalInput")
    wkt = nc.dram_tensor("wkt", [C, C], BF16, kind="ExternalInput")
    wvt = nc.dram_tensor("wvt", [C, C], BF16, kind="ExternalInput")
    wot = nc.dram_tensor("wot", [C, C], BF16, kind="ExternalInput")
    out = nc.dram_tensor("out", [M, C], BF16, kind="ExternalOutput")

    # [C, X] dram views chunked to [P, KC, X]
    xt_v = xt[:, :].rearrange("(kc p) m -> p kc m", p=P)
    yt_v = yt[:, :].rearrange("(kc p) t -> p kc t", p=P)
    wq_v = wqt[:, :].rearrange("(kc p) n -> p kc n", p=P)
    wk_v = wkt[:, :].rearrange("(kc p) n -> p kc n", p=P)
    wv_v = wvt[:, :].rearrange("(kc p) n -> p kc n", p=P)
    wo_v = wot[:, :].rearrange("(kc p) n -> p kc n", p=P)

    with tile.TileContext(nc) as tc:
        with (
            tc.tile_pool(name="wq", bufs=1) as wq_pool,
            tc.tile_pool(name="wo", bufs=1) as wo_pool,
            tc.tile_pool(name="kt", bufs=1) as kt_pool,
            tc.tile_pool(name="vv", bufs=1) as v_pool,
        ):
            wq_sb = wq_pool.tile([P, KC, C], BF16)
            wo_sb = wo_pool.tile([P, KC, C], BF16)
            kt_sb = kt_pool.tile([P, KC, TP], BF16)
            v_sb = v_pool.tile([TP, H, DV], BF16)

            nc.sync.dma_start(wq_sb[:], wq_v[:])

            # ---- prologue: K^T and V from text; load Wk/Wv/Wo ----
            with (
                tc.tile_pool(name="wkv", bufs=1) as wkv_pool,
                tc.tile_pool(name="yt", bufs=1) as yt_pool,
                tc.tile_pool(name="pspro", bufs=2, space="PSUM") as ps_pro,
            ):
                yt_sb = yt_pool.tile([P, KC, TP], BF16)
                nc.vector.memset(yt_sb[:], 0.0)
                nc.sync.dma_start(yt_sb[:, :, :T], yt_v[:])

                wk_sb = wkv_pool.tile([P, KC, C], BF16, tag="wkv")
                nc.scalar.dma_start(wk_sb[:], wk_v[:])
                # K^T [C, T]: chunk nc_ holds rows 128*nc_..128*nc_+128
                for nc_ in range(KC):
                    psk_full = ps_pro.tile([P, MB], F32, tag="pro", name="psk")
                    psk = psk_full[:, :TP]
                    for kc in range(KC):
                        nc.tensor.matmul(
                            psk[:],
                            wk_sb[:, kc, nc_ * P : (nc_ + 1) * P],
                            yt_sb[:, kc, :],
                            start=(kc == 0),
                            stop=(kc == KC - 1),
                        )
                    nc.vector.tensor_copy(out=kt_sb[:, nc_, :], in_=psk[:])

                wv_sb = wkv_pool.tile([P, KC, C], BF16, tag="wkv")
                nc.scalar.dma_start(wv_sb[:], wv_v[:])
                # V natural [T, C] written per 512-wide column slab into
                # the strided per-head layout v_sb[t, h, 0:64]; pad rows
                # 77:80 stay zero.
                nc.vector.memset(v_sb[:], 0.0)
                for half in range(2):
                    psv_full = ps_pro.tile([P, MB], F32, tag="pro", name="psv")
                    psv = psv_full[:T, :]
                    for kc in range(KC):
                        nc.tensor.matmul(
                            psv[:],
                            yt_sb[:, kc, :T],
                            wv_sb[:, kc, half * MB : (half + 1) * MB],
                            start=(kc == 0),
                            stop=(kc == KC - 1),
                        )
                    nc.vector.tensor_copy(
                        out=v_sb[:T, half * 8 : (half + 1) * 8, 0:D],
                        in_=psv[:].rearrange("t (h d) -> t h d", d=D),
                    )
                nc.vector.memset(v_sb[:T, :, D : D + 1], 1.0)

            nc.scalar.dma_start(wo_sb[:], wo_v[:])

            # ---- main pipeline over m-blocks ----
            from contextlib import ExitStack

            with ExitStack() as _st:
                xt_pool = _st.enter_context(tc.tile_pool(name="xt", bufs=2))
                qt_pool = _st.enter_context(tc.tile_pool(name="qt", bufs=2))
                ot_pool = _st.enter_context(tc.tile_pool(name="ot", bufs=2))
                ou_pool = _st.enter_context(tc.tile_pool(name="ou", bufs=2))
                es_pool = _st.enter_context(tc.tile_pool(name="es", bufs=2))
                rc_pool = _st.enter_context(tc.tile_pool(name="rc", bufs=2))
                ln_pool = _st.enter_context(tc.tile_pool(name="ln", bufs=3))
                rb_pool = _st.enter_context(tc.tile_pool(name="rb", bufs=2))
                ob_pool = _st.enter_context(tc.tile_pool(name="ob", bufs=3))
                ps_q = _st.enter_context(tc.tile_pool(name="psq", bufs=2, space="PSUM"))
                ps_s = _st.enter_context(tc.tile_pool(name="pss", bufs=2, space="PSUM"))
                ps_o = _st.enter_context(tc.tile_pool(name="pso", bufs=2, space="PSUM"))
                ps_out = _st.enter_context(tc.tile_pool(name="psout", bufs=2, space="PSUM"))
                for j in [jj for _ in range(repeat) for jj in range(nblk)]:
                    xt_t = xt_pool.tile([P, KC, MB], BF16, tag="xt")
                    nc.sync.dma_start(
                        xt_t[:], xt_v[:, :, j * MB : (j + 1) * MB]
                    )

                    # Q^T chunks for this block
                    qt_t = qt_pool.tile([P, KC, MB], BF16, tag="qt")
                    for nc_ in range(KC):
                        psq = ps_q.tile([P, MB], F32, tag="psq")
                        for kc in range(KC):
                            nc.tensor.matmul(
                                psq[:],
                                wq_sb[:, kc, nc_ * P : (nc_ + 1) * P],
                                xt_t[:, kc, :],
                                start=(kc == 0),
                                stop=(kc == KC - 1),
                            )
                        nc.vector.tensor_copy(out=qt_t[:, nc_, :], in_=psq[:])

                    # attention phase 1: scores + exp for all heads
                    es_t = es_pool.tile([TP, H, MB], BF16, tag="es")
                    for h in range(H):
                        jc, hf = divmod(h, 2)
                        lo, hi = 64 * hf, 64 * hf + 64
                        pss = ps_s.tile([TP, MB], F32, tag="pss")
                        nc.tensor.matmul(
                            pss[:],
                            kt_sb[lo:hi, jc, :],
                            qt_t[lo:hi, jc, :],
                            start=True,
                            stop=True,
                        )
                        nc.scalar.activation(
                            es_t[:, h, :], pss[:], AF.Exp, scale=SCALE
                        )

                    # phase 2: AV into ou_t (unnormalized O'), selector
                    # matmuls accumulate all 16 denominators into psden.
                    ot_t = ot_pool.tile([P, KC, MB], BF16, tag="ot")
                    ou_t = ou_pool.tile([P, KC, MB], BF16, tag="ou")
                    # rcb[0, hf, jc, :] = 1/den of head h = 2*jc + hf via
                    # exp(-ln(den)): Ln/Exp share one ACT table set (see
                    # the get_activation_tables patch above), so no table
                    # reloads.
                    rcb = rc_pool.tile([1, 2, KC, MB], BF16, tag="rc")
                    for h in range(H):
                        jc, hf = divmod(h, 2)
                        lo, hi = 64 * hf, 64 * hf + 64
                        pso = ps_o.tile([DV, MB], F32, tag="pso")
                        nc.tensor.matmul(
                            pso[:], v_sb[:, h, :], es_t[:, h, :],
                            start=True, stop=True,
                        )
                        # softmax denominator lives in PSUM row D
                        lnd = ln_pool.tile([1, MB], F32, tag="lnd")
                        nc.scalar.activation(
                            lnd[:], pso[D : D + 1, :], AF.Ln
                        )
                        nc.scalar.activation(
                            rcb[:, hf, jc, :], lnd[:],
                            AF.Exp, scale=-1.0,
                        )
                        nc.vector.tensor_copy(
                            out=ou_t[lo:hi, jc, :], in_=pso[0:D, :]
                        )

                    # batched partition-broadcast of the reciprocals:
                    # rb[64*hf + p, jc, m] = rcb[0, hf, jc, m]
                    rb = rb_pool.tile([P, KC, MB], BF16, tag="rb")
                    for hf in range(2):
                        nc.scalar.dma_start(
                            rb[64 * hf : 64 * hf + 64, :, :],
                            rcb[0:1, hf, None, :, :].to_broadcast(
                                (1, 64, KC, MB)
                            ),
                        )
                    for jh in range(2):
                        js = jh * (KC // 2)
                        je = js + KC // 2
                        nc.vector.tensor_tensor(
                            ot_t[:, js:je, :],
                            ou_t[:, js:je, :],
                            rb[:, js:je, :],
                            MULT,
                        )

                    # output projection, natural [m, n] layout
                    for mi in range(MSUB):
                        ob = ob_pool.tile([P, 2, MB], BF16, tag="ob")
                        for nh in range(2):
                            pst = ps_out.tile([P, MB], F32, tag="psout")
                            for cc in range(KC):
                                nc.tensor.matmul(
                                    pst[:],
                                    ot_t[:, cc, mi * P : (mi + 1) * P],
                                    wo_sb[:, cc, nh * MB : (nh + 1) * MB],
                                    start=(cc == 0),
                                    stop=(cc == KC - 1),
                                )
                            nc.vector.tensor_copy(out=ob[:, nh, :], in_=pst[:])
                        nc.sync.dma_start(
                            out[j * MB + mi * P : j * MB + (mi + 1) * P, :],
                            ob[:],
                        )
    nc.finalize()
    return nc


def _get_nc(repeat: int = 1):
    global _CACHED_NC
    if _CACHED_NC is None:
        _CACHED_NC = {}
    if repeat not in _CACHED_NC:
        _CACHED_NC[repeat] = _build(repeat)
    return _CACHED_NC[repeat]


def _np_bf16():
    return np.dtype(mybir.dt.np(BF16))


def make_in_maps(video_features, text_features, Wq, Wk, Wv, Wo):
    bf16 = _np_bf16()
    video_features = np.asarray(video_features, dtype=np.float32)
    text_features = np.asarray(text_features, dtype=np.float32)
    wqt = np.ascontiguousarray(np.asarray(Wq, dtype=np.float32).T).astype(bf16)
    wkt = np.ascontiguousarray(np.asarray(Wk, dtype=np.float32).T).astype(bf16)
    wvt = np.ascontiguousarray(np.asarray(Wv, dtype=np.float32).T).astype(bf16)
    wot = np.ascontiguousarray(np.asarray(Wo, dtype=np.float32).T).astype(bf16)

    in_maps = []
    for c in range(8):
        b, half = divmod(c, 2)
        xs = video_features[b, half * M : (half + 1) * M, :]  # [M, C]
        in_maps.append(
            {
                "xt": np.ascontiguousarray(xs.T).astype(bf16),    # [C, M]
                "yt": np.ascontiguousarray(text_features[b].T).astype(bf16),
                "wqt": wqt,
                "wkt": wkt,
                "wvt": wvt,
                "wot": wot,
            }
        )
    return in_maps


def kernel(video_features, text_features, Wq, Wk, Wv, Wo, **_unused):
    in_maps = make_in_maps(video_features, text_features, Wq, Wk, Wv, Wo)
    res = run_bass_kernel_spmd(_get_nc(), in_maps, core_ids=list(range(8)))
    outf = np.empty((B, T_V, C), dtype=np.float32)
    for c in range(8):
        b, half = divmod(c, 2)
        outf[b, half * M : (half + 1) * M, :] = (
            res.results[c]["out"].astype(np.float32)
        )
    return outf
